# revision 6
# baseline (speedup 1.0000x reference)
"""Trainium2 Bass kernel for nn_Attention_49409303773358.

Masked ViT attention (B=4, N=1025, dim=768, 12 heads x d=64) with mask
propagation, sharded over 8 NeuronCores as 4 batches x 2 head-groups
(6 heads each).  Each core computes its batch's qkv projection for its 6
heads, the mask propagation, per-head attention, and a *partial* output
projection over its 384 inner dims; the host sums the two partials per
batch (valid because the output projection is linear in the inner dim).

Compute dtype: bf16 operands with f32 PSUM accumulation.

Device layout: "transposed" feature-on-partition layout throughout
([dim, tokens]); tokens padded 1025 -> 1152 = 9*128 on the host.
Softmax runs without max-subtraction (logits are O(1) for this problem's
scale), denominators via an all-ones stationary matmul which also
broadcasts them across partitions.
"""

import sys

for p in ("/opt/trn_rl_repo",):
    if p not in sys.path:
        sys.path.insert(0, p)

import numpy as np
import ml_dtypes

import concourse.bass as bass
import concourse.tile as tile
from concourse import bacc, mybir
from concourse.bass_utils import run_bass_kernel_spmd

BF16 = ml_dtypes.bfloat16

# Problem constants (per core)
DIM = 768          # model dim
DC = 6             # dim chunks of 128
NTOK = 1025        # real tokens (CLS + 1024 patches)
NPAD = 1152        # padded tokens = 9*128
TC = 9             # token chunks of 128
QT = 384           # query tile width
NQT = 3            # query tiles (3*384 = 1152)
HEADS = 6          # heads per core
HD = 64            # head dim
INNER = 384        # inner dims per core (6*64)
IC = 3             # inner chunks of 128
SCALE = 0.125      # d ** -0.5

F32 = mybir.dt.float32
BF = mybir.dt.bfloat16


def _emit(nc, tc_ctx, d):
    """Emit the whole per-core program under a TileContext."""
    tc = tc_ctx
    AX = mybir.AxisListType
    ALU = mybir.AluOpType
    ACTF = mybir.ActivationFunctionType

    from contextlib import ExitStack
    with ExitStack() as ctx:
        singles = ctx.enter_context(tc.tile_pool(name="singles", bufs=1))
        exp_pool = ctx.enter_context(tc.tile_pool(name="exps", bufs=2))
        mult_pool = ctx.enter_context(tc.tile_pool(name="mults", bufs=2))
        small_pool = ctx.enter_context(tc.tile_pool(name="smalls", bufs=2))
        strip_pool = ctx.enter_context(tc.tile_pool(name="strips", bufs=2, space="PSUM"))
        acc_pool = ctx.enter_context(tc.tile_pool(name="accs", bufs=2, space="PSUM"))
        dram_pool = ctx.enter_context(tc.tile_pool(name="drams", bufs=1, space="DRAM"))
        # ---- persistent SBUF tensors ----
        ones128 = singles.tile([128, 128], BF, tag="ones128")
        nc.vector.memset(ones128[:, :], 1.0)

        s_xT = singles.tile([128, DC, NPAD], BF, tag="s_xT")
        s_maskT = singles.tile([128, DC, NPAD], BF, tag="s_maskT")
        s_wqkT = singles.tile([128, DC, 768], BF, tag="s_wqkT")
        s_wqkTa = singles.tile([128, DC, 768], BF, tag="s_wqkTa")
        s_wvT = singles.tile([128, DC, INNER], BF, tag="s_wvT")
        s_wvTa = singles.tile([128, DC, INNER], BF, tag="s_wvTa")
        s_woT = singles.tile([128, IC, 768], BF, tag="s_woT")
        s_woTa = singles.tile([128, IC, 768], BF, tag="s_woTa")

        qqkkT = singles.tile([128, DC, NPAD], BF, tag="qqkkT")   # qq chunks 0..2, kk 3..5
        qkmT = singles.tile([128, DC, NPAD], BF, tag="qkmT")     # q_m chunks 0..2, k_m 3..5
        vvm = singles.tile([128, TC, 768], BF, tag="vvm")        # per head h: [vv_h 64 | vm_h 64]
        ohT = singles.tile([128, IC, NPAD], BF, tag="ohT")
        mhT = singles.tile([128, IC, NPAD], BF, tag="mhT")
        updated = singles.tile([128, NPAD], BF, tag="updated")   # row-replicated updated mask
        urow = singles.tile([1, NPAD], BF, tag="urow")
        vrec_cols = singles.tile([128, 3], BF, tag="vrec_cols")  # 1/(1e-6+max) of v_m, col-major
        vrec_row = singles.tile([1, INNER], BF, tag="vrec_row")
        bcast_vrec = singles.tile([128, INNER], BF, tag="bcast_vrec")

        vrec_dram = dram_pool.tile([INNER], BF, tag="vrec_dram")

        # ---- input DMAs ----
        def load_T(sb, dram_h, chunks, width):
            ap = dram_h.ap().rearrange("(c p) n -> p c n", p=128)
            for c in range(chunks):
                nc.sync.dma_start(out=sb[:, c, :], in_=ap[:, c, :])

        load_T(s_xT, d["xT"], DC, NPAD)
        load_T(s_maskT, d["maskT"], DC, NPAD)
        load_T(s_wqkT, d["wqkT"], DC, 768)
        load_T(s_wqkTa, d["wqkTa"], DC, 768)
        load_T(s_wvT, d["wvT"], DC, INNER)
        load_T(s_wvTa, d["wvTa"], DC, INNER)
        load_T(s_woT, d["woT"], IC, 768)
        load_T(s_woTa, d["woTa"], IC, 768)

        # ---- updated = (sum_d mask > 0), broadcast to all partitions ----
        for qt in range(NQT):
            ps = acc_pool.tile([128, QT], F32, tag="acc")
            for kc in range(DC):
                nc.tensor.matmul(
                    ps[0:1, :], ones128[:, 0:1], s_maskT[:, kc, qt * QT:(qt + 1) * QT],
                    start=(kc == 0), stop=(kc == DC - 1),
                )
            nc.vector.tensor_copy(urow[0:1, qt * QT:(qt + 1) * QT], ps[0:1, :])
        for qt in range(NQT):
            ps = acc_pool.tile([128, QT], F32, tag="acc")
            nc.tensor.matmul(
                ps[:, :], ones128[0:1, :], urow[0:1, qt * QT:(qt + 1) * QT],
                start=True, stop=True,
            )
            nc.vector.tensor_scalar(
                out=updated[:, qt * QT:(qt + 1) * QT], in0=ps[:, :],
                scalar1=0.0, scalar2=None, op0=ALU.is_gt,
            )
        nc.vector.memset(updated[:, 0:1], 1.0)  # CLS token always updated

        # ---- mask propagation projection (transposed layout) ----
        # m-chunks 0..5 -> q_m / k_m (stored + normalized); 6..8 -> v_m (max only)
        for m in range(9):
            strip = strip_pool.tile([128, NQT, 512], F32, tag="strip")
            for qt in range(NQT):
                for kc in range(DC):
                    if m < 6:
                        lhsT = s_wqkTa[:, kc, m * 128:(m + 1) * 128]
                    else:
                        lhsT = s_wvTa[:, kc, (m - 6) * 128:(m - 5) * 128]
                    nc.tensor.matmul(
                        strip[:, qt, 0:QT], lhsT,
                        s_maskT[:, kc, qt * QT:(qt + 1) * QT],
                        start=(kc == 0), stop=(kc == DC - 1),
                    )
            if m < 6:
                # raw copy -> SBUF bf16, then max / reciprocal / in-place norm
                for qt in range(NQT):
                    nc.vector.tensor_copy(
                        qkmT[:, m, qt * QT:(qt + 1) * QT], strip[:, qt, 0:QT])
                rmax = small_pool.tile([128, 1], F32, tag="rmax")
                nc.vector.reduce_max(rmax[:, :], qkmT[:, m, :], axis=AX.X)
                nc.vector.tensor_scalar_add(rmax[:, :], rmax[:, :], 1e-6)
                rrec = small_pool.tile([128, 1], F32, tag="rrec")
                nc.vector.reciprocal(rrec[:, :], rmax[:, :])
                nc.vector.tensor_scalar_mul(qkmT[:, m, :], qkmT[:, m, :], rrec[:, :])
                nc.vector.memset(qkmT[:, m, 0:1], 1.0)  # CLS -> 1
            else:
                rmax = small_pool.tile([128, 1], F32, tag="rmax")
                nc.vector.reduce_max(rmax[:, :], strip[:, :, 0:QT], axis=AX.XY)
                nc.vector.tensor_scalar_add(rmax[:, :], rmax[:, :], 1e-6)
                rrec = small_pool.tile([128, 1], F32, tag="rrec")
                nc.vector.reciprocal(rrec[:, :], rmax[:, :])
                nc.vector.tensor_copy(vrec_cols[:, m - 6:m - 5], rrec[:, :])

        # vrec columns [128, 3] -> row [1, 384] via DRAM round-trip
        nc.sync.dma_start(
            out=vrec_dram[:].rearrange("(m p) -> p m", p=128), in_=vrec_cols[:, :])
        nc.sync.dma_start(
            out=vrec_row[0:1, :], in_=vrec_dram[:].rearrange("(a n) -> a n", a=1))
        psb = acc_pool.tile([128, INNER], F32, tag="acc")
        nc.tensor.matmul(psb[:, :], ones128[0:1, :], vrec_row[0:1, :],
                         start=True, stop=True)
        nc.vector.tensor_copy(bcast_vrec[:, :], psb[:, :])

        # ---- x projection for q, k (transposed layout), fused with q_m/k_m mult ----
        for m in range(6):
            strip = strip_pool.tile([128, NQT, 512], F32, tag="strip")
            for qt in range(NQT):
                for kc in range(DC):
                    nc.tensor.matmul(
                        strip[:, qt, 0:QT], s_wqkT[:, kc, m * 128:(m + 1) * 128],
                        s_xT[:, kc, qt * QT:(qt + 1) * QT],
                        start=(kc == 0), stop=(kc == DC - 1),
                    )
            for qt in range(NQT):
                nc.vector.tensor_mul(
                    qqkkT[:, m, qt * QT:(qt + 1) * QT],
                    strip[:, qt, 0:QT],
                    qkmT[:, m, qt * QT:(qt + 1) * QT],
                )

        # ---- v / v_m in token layout, fused products into vvm ----
        for tg in range(3):
            psV = strip_pool.tile([128, NQT, 512], F32, tag="strip")
            psVm = strip_pool.tile([128, NQT, 512], F32, tag="strip")
            for t3 in range(3):
                tok = tg * 3 + t3
                for kc in range(DC):
                    nc.tensor.matmul(
                        psV[:, t3, 0:INNER], s_xT[:, kc, tok * 128:(tok + 1) * 128],
                        s_wvT[:, kc, :],
                        start=(kc == 0), stop=(kc == DC - 1),
                    )
                for kc in range(DC):
                    nc.tensor.matmul(
                        psVm[:, t3, 0:INNER], s_maskT[:, kc, tok * 128:(tok + 1) * 128],
                        s_wvTa[:, kc, :],
                        start=(kc == 0), stop=(kc == DC - 1),
                    )
            for t3 in range(3):
                tok = tg * 3 + t3
                # vm (normalized) -> odd 64-blocks of vvm
                vm_dst = vvm[:, tok, :].rearrange("p (h two hd) -> p h two hd", two=2, hd=HD)[:, :, 1, :]
                vm_src = psVm[:, t3, 0:INNER].rearrange("p (h hd) -> p h hd", hd=HD)
                rec_v = bcast_vrec[:, :].rearrange("p (h hd) -> p h hd", hd=HD)
                nc.vector.tensor_mul(vm_dst, vm_src, rec_v)
                if tok == 0:
                    # CLS row of v_m is exactly 1
                    nc.vector.memset(vvm[0:1, 0, :].rearrange(
                        "p (h two hd) -> p h two hd", two=2, hd=HD)[:, :, 1, :], 1.0)
                # vv = vm * v -> even 64-blocks of vvm
                vv_dst = vvm[:, tok, :].rearrange("p (h two hd) -> p h two hd", two=2, hd=HD)[:, :, 0, :]
                v_src = psV[:, t3, 0:INNER].rearrange("p (h hd) -> p h hd", hd=HD)
                nc.vector.tensor_mul(vv_dst, vm_dst, v_src)

        # ---- attention per head ----
        for h in range(HEADS):
            hb = (h % 2) * 64          # partition base within chunk
            hc = h // 2                # chunk index
            expT = exp_pool.tile([128, TC, NPAD], BF, tag="expT")
            # last key chunk holds only one real key (token 1024): zero the
            # whole chunk first, then let exp write just row 0.
            nc.vector.memset(expT[:, TC - 1, :], 0.0)
            for kc in range(TC):
                strip = strip_pool.tile([128, NQT, 512], F32, tag="strip")
                for qt in range(NQT):
                    nc.tensor.matmul(
                        strip[:, qt, 0:QT],
                        qqkkT[hb:hb + HD, 3 + hc, kc * 128:(kc + 1) * 128],
                        qqkkT[hb:hb + HD, hc, qt * QT:(qt + 1) * QT],
                        start=True, stop=True,
                    )
                if kc == TC - 1:
                    nc.scalar.activation(
                        out=expT[0:1, kc, :], in_=strip[0:1, :, 0:QT],
                        func=ACTF.Exp, scale=SCALE,
                    )
                else:
                    nc.scalar.activation(
                        out=expT[:, kc, :], in_=strip[:, :, 0:QT],
                        func=ACTF.Exp, scale=SCALE,
                    )

            multb = mult_pool.tile([128, NPAD], BF, tag="multb")
            for qt in range(NQT):
                dacc = acc_pool.tile([128, QT], F32, tag="acc")
                for kc in range(TC):
                    nc.tensor.matmul(
                        dacc[:, :], ones128[:, :],
                        expT[:, kc, qt * QT:(qt + 1) * QT],
                        start=(kc == 0), stop=(kc == TC - 1),
                    )
                # 1/denom as exp(-ln(denom)) on the scalar engine — the DVE
                # iterative reciprocal is ~8 cyc/elem and was a bottleneck.
                rf32 = small_pool.tile([128, QT], F32, tag="rf32")
                nc.scalar.activation(out=rf32[:, :], in_=dacc[:, :], func=ACTF.Ln)
                nc.scalar.activation(out=multb[:, qt * QT:(qt + 1) * QT],
                                     in_=rf32[:, :], func=ACTF.Exp, scale=-1.0)
                nc.vector.tensor_mul(
                    multb[:, qt * QT:(qt + 1) * QT],
                    multb[:, qt * QT:(qt + 1) * QT],
                    updated[:, qt * QT:(qt + 1) * QT])
            for qt in range(NQT):
                aacc = acc_pool.tile([128, QT], F32, tag="acc")
                for kc in range(TC):
                    nc.tensor.matmul(
                        aacc[:, :], vvm[:, kc, h * 128:(h + 1) * 128],
                        expT[:, kc, qt * QT:(qt + 1) * QT],
                        start=(kc == 0), stop=(kc == TC - 1),
                    )
                nc.vector.tensor_mul(
                    ohT[hb:hb + HD, hc, qt * QT:(qt + 1) * QT],
                    aacc[0:HD, :], multb[0:HD, qt * QT:(qt + 1) * QT])
                nc.vector.tensor_mul(
                    mhT[hb:hb + HD, hc, qt * QT:(qt + 1) * QT],
                    aacc[HD:128, :], multb[HD:128, qt * QT:(qt + 1) * QT])

        # ---- final projections (partial over this core's 384 inner dims) ----
        outd = d["out"]
        for t, (wmat, src) in enumerate(((s_woT, ohT), (s_woTa, mhT))):
            for m in range(6):
                strip = strip_pool.tile([128, NQT, 512], F32, tag="strip")
                for qt in range(NQT):
                    for ic in range(IC):
                        nc.tensor.matmul(
                            strip[:, qt, 0:QT], wmat[:, ic, m * 128:(m + 1) * 128],
                            src[:, ic, qt * QT:(qt + 1) * QT],
                            start=(ic == 0), stop=(ic == IC - 1),
                        )
                stage = mult_pool.tile([128, NPAD], F32, tag="ostage")
                nc.vector.tensor_copy(stage[:, :].rearrange("p (q n) -> p q n", n=QT),
                                      strip[:, :, 0:QT])
                nc.sync.dma_start(
                    out=outd.ap()[t, m * 128:(m + 1) * 128, :],
                    in_=stage[:, :],
                )


_CACHED = None


def _build():
    global _CACHED
    if _CACHED is not None:
        return _CACHED
    nc = bacc.Bacc("TRN2", target_bir_lowering=False, debug=False, num_devices=8)
    d = {}
    d["xT"] = nc.dram_tensor("xT", [DIM, NPAD], BF, kind="ExternalInput")
    d["maskT"] = nc.dram_tensor("maskT", [DIM, NPAD], BF, kind="ExternalInput")
    d["wqkT"] = nc.dram_tensor("wqkT", [DIM, 768], BF, kind="ExternalInput")
    d["wqkTa"] = nc.dram_tensor("wqkTa", [DIM, 768], BF, kind="ExternalInput")
    d["wvT"] = nc.dram_tensor("wvT", [DIM, INNER], BF, kind="ExternalInput")
    d["wvTa"] = nc.dram_tensor("wvTa", [DIM, INNER], BF, kind="ExternalInput")
    d["woT"] = nc.dram_tensor("woT", [INNER, 768], BF, kind="ExternalInput")
    d["woTa"] = nc.dram_tensor("woTa", [INNER, 768], BF, kind="ExternalInput")
    d["out"] = nc.dram_tensor("out", [2, DIM, NPAD], F32, kind="ExternalOutput")

    with tile.TileContext(nc) as tcx:
        _emit(nc, tcx, d)
    nc.compile()
    _CACHED = nc
    return nc


def _host_prep(x, mask, w_qkv, w_out):
    """Build the 8 per-core input maps (bf16, transposed + padded)."""
    B = x.shape[0]
    in_maps = []
    for core in range(8):
        b, g = core // 2, core % 2
        sl = slice(g * INNER, (g + 1) * INNER)

        xT = np.zeros((DIM, NPAD), dtype=BF16)
        xT[:, :NTOK] = x[b].T.astype(BF16)
        maskT = np.zeros((DIM, NPAD), dtype=BF16)
        maskT[:, 1:NTOK] = mask[b].T.astype(BF16)

        w_q = w_qkv[sl]                      # [384, 768]
        w_k = w_qkv[768 + g * INNER: 768 + (g + 1) * INNER]
        w_v = w_qkv[1536 + g * INNER: 1536 + (g + 1) * INNER]
        wqk = np.concatenate([w_q, w_k], axis=0)          # [768, 768]
        wqkT = np.ascontiguousarray(wqk.T).astype(BF16)
        wqkTa = np.abs(wqkT).astype(BF16)
        wvT = np.ascontiguousarray(w_v.T).astype(BF16)     # [768, 384]
        wvTa = np.abs(wvT).astype(BF16)
        woT = np.ascontiguousarray(w_out[:, sl].T).astype(BF16)   # [384, 768]
        woTa = np.abs(woT).astype(BF16)

        in_maps.append({
            "xT": xT, "maskT": maskT,
            "wqkT": wqkT, "wqkTa": wqkTa,
            "wvT": wvT, "wvTa": wvTa,
            "woT": woT, "woTa": woTa,
        })
    return in_maps


def run(inputs, trace=False):
    """Run on hardware; returns ((out, m), exec_time_ns)."""
    x = np.asarray(inputs["x"], dtype=np.float32)
    mask = np.asarray(inputs["mask"], dtype=np.float32)
    w_qkv = np.asarray(inputs["w_qkv"], dtype=np.float32)
    w_out = np.asarray(inputs["w_out"], dtype=np.float32)
    b_out = np.asarray(inputs["b_out"], dtype=np.float32)

    nc = _build()
    in_maps = _host_prep(x, mask, w_qkv, w_out)
    res = run_bass_kernel_spmd(nc, in_maps, list(range(8)), trace=trace)

    B = x.shape[0]
    outs, ms = [], []
    for b in range(B):
        p0 = np.asarray(res.results[2 * b]["out"], dtype=np.float32)
        p1 = np.asarray(res.results[2 * b + 1]["out"], dtype=np.float32)
        s = p0 + p1                                  # [2, 768, 1152]
        outs.append(s[0].T[:NTOK] + b_out)           # [1025, 768]
        ms.append(s[1].T[1:NTOK])                    # [1024, 768]
    out = np.stack(outs).astype(np.float32)
    m = np.stack(ms).astype(np.float32)
    return (out, m), res.exec_time_ns


def kernel(**inputs):
    (out, m), _ = run(inputs, trace=False)
    return (out, m)


# revision 13
# speedup vs baseline: 1.2676x; 1.2676x over previous
"""Trainium2 Bass kernel for nn_Attention_49409303773358.

Masked ViT attention (B=4, N=1025, dim=768, 12 heads x d=64) with mask
propagation, sharded over 8 NeuronCores as 4 batches x 2 head-groups
(6 heads each).  Each core computes its batch's qkv projection for its 6
heads, the mask propagation, per-head attention, and a *partial* output
projection over its 384 inner dims; the host sums the two partials per
batch (valid because the output projection is linear in the inner dim).

Compute dtype: bf16 operands with f32 PSUM accumulation.

Device layout: "transposed" feature-on-partition layout throughout
([dim, tokens]); tokens padded 1025 -> 1152 = 9*128 on the host.
Softmax runs without max-subtraction (logits are O(1) for this problem's
scale), denominators via an all-ones stationary matmul which also
broadcasts them across partitions.
"""

import sys

for p in ("/opt/trn_rl_repo",):
    if p not in sys.path:
        sys.path.insert(0, p)

import numpy as np
import ml_dtypes

import concourse.bass as bass
import concourse.tile as tile
from concourse import bacc, mybir
from concourse.bass_utils import run_bass_kernel_spmd


def _patch_act_tables():
    """Prefer the activation-table set that holds BOTH Exp and Ln so the
    per-head Ln (softmax denominator reciprocal) doesn't force an
    ACT_TABLE_LOAD (~2.7us) between every Exp/Ln alternation."""
    import concourse.bacc as _bacc_mod
    orig = _bacc_mod.get_activation_tables

    def filtered(arch):
        t = orig(arch)
        pref = "natural_log_exp_and_others"
        if pref not in t:
            return t
        # Keep dict order (index == act_func_set_id, walrus interprets it
        # against act_info.json) but hide Exp/Ln from every other set so the
        # combined set is the only candidate for both.
        exp_ln = {k for k in t[pref]
                  if k.name in ("Exp", "Ln")}
        out = {}
        for name, fns in t.items():
            out[name] = fns if name == pref else (fns - exp_ln)
        return out

    _bacc_mod.get_activation_tables = filtered


_patch_act_tables()

BF16 = ml_dtypes.bfloat16

# Problem constants (per core)
DIM = 768          # model dim
DC = 6             # dim chunks of 128
NTOK = 1025        # real tokens (CLS + 1024 patches)
NPAD = 1152        # padded tokens = 9*128
TC = 9             # token chunks of 128
QT = 384           # query tile width
NQT = 3            # query tiles (3*384 = 1152)
HEADS = 6          # heads per core
HD = 64            # head dim
INNER = 384        # inner dims per core (6*64)
IC = 3             # inner chunks of 128
SCALE = 0.125      # d ** -0.5

F32 = mybir.dt.float32
BF = mybir.dt.bfloat16


def _emit(nc, tc_ctx, d):
    """Emit the whole per-core program under a TileContext."""
    tc = tc_ctx
    AX = mybir.AxisListType
    ALU = mybir.AluOpType
    ACTF = mybir.ActivationFunctionType

    from contextlib import ExitStack
    with ExitStack() as ctx:
        singles = ctx.enter_context(tc.tile_pool(name="singles", bufs=1))
        exp_pool = ctx.enter_context(tc.tile_pool(name="exps", bufs=2))
        mult_pool = ctx.enter_context(tc.tile_pool(name="mults", bufs=2))
        small_pool = ctx.enter_context(tc.tile_pool(name="smalls", bufs=2))
        strip_pool = ctx.enter_context(tc.tile_pool(name="strips", bufs=2, space="PSUM"))
        acc_pool = ctx.enter_context(tc.tile_pool(name="accs", bufs=2, space="PSUM"))
        dram_pool = ctx.enter_context(tc.tile_pool(name="drams", bufs=1, space="DRAM"))
        # ---- persistent SBUF tensors ----
        ones128 = singles.tile([128, 128], BF, tag="ones128")
        nc.vector.memset(ones128[:, :], 1.0)

        s_xT = singles.tile([128, DC, NPAD], BF, tag="s_xT")
        s_maskT = singles.tile([128, DC, NPAD], BF, tag="s_maskT")
        s_wqkT = singles.tile([128, DC, 768], BF, tag="s_wqkT")
        s_wqkTa = singles.tile([128, DC, 768], BF, tag="s_wqkTa")
        s_wvT = singles.tile([128, DC, INNER], BF, tag="s_wvT")
        s_wvTa = singles.tile([128, DC, INNER], BF, tag="s_wvTa")
        s_woT = singles.tile([128, IC, 768], BF, tag="s_woT")
        s_woTa = singles.tile([128, IC, 768], BF, tag="s_woTa")

        qqkkT = singles.tile([128, DC, NPAD], BF, tag="qqkkT")   # qq chunks 0..2, kk 3..5
        qkmT = singles.tile([128, DC, NPAD], BF, tag="qkmT")     # q_m chunks 0..2, k_m 3..5
        vvm = singles.tile([128, TC, 768], BF, tag="vvm")        # per head h: [vv_h 64 | vm_h 64]
        ohT = singles.tile([128, IC, NPAD], BF, tag="ohT")
        mhT = singles.tile([128, IC, NPAD], BF, tag="mhT")
        updated = singles.tile([128, NPAD], BF, tag="updated")   # row-replicated updated mask
        urow = singles.tile([1, NPAD], BF, tag="urow")
        vrec_cols = singles.tile([128, 3], BF, tag="vrec_cols")  # 1/(1e-6+max) of v_m, col-major
        vrec_row = singles.tile([1, INNER], BF, tag="vrec_row")
        bcast_vrec = singles.tile([128, INNER], BF, tag="bcast_vrec")

        vrec_dram = dram_pool.tile([INNER], BF, tag="vrec_dram")

        # ---- input DMAs ----
        def load_T(sb, dram_h, chunks, width):
            ap = dram_h.ap().rearrange("(c p) n -> p c n", p=128)
            for c in range(chunks):
                nc.sync.dma_start(out=sb[:, c, :], in_=ap[:, c, :])

        load_T(s_xT, d["xT"], DC, NPAD)
        load_T(s_maskT, d["maskT"], DC, NPAD)
        load_T(s_wqkT, d["wqkT"], DC, 768)
        load_T(s_wqkTa, d["wqkTa"], DC, 768)
        load_T(s_wvT, d["wvT"], DC, INNER)
        load_T(s_wvTa, d["wvTa"], DC, INNER)
        load_T(s_woT, d["woT"], IC, 768)
        load_T(s_woTa, d["woTa"], IC, 768)

        # ---- updated = (sum_d mask > 0), broadcast to all partitions ----
        for qt in range(NQT):
            ps = acc_pool.tile([128, QT], F32, tag="acc")
            for kc in range(DC):
                nc.tensor.matmul(
                    ps[0:1, :], ones128[:, 0:1], s_maskT[:, kc, qt * QT:(qt + 1) * QT],
                    start=(kc == 0), stop=(kc == DC - 1),
                )
            nc.vector.tensor_copy(urow[0:1, qt * QT:(qt + 1) * QT], ps[0:1, :])
        for qt in range(NQT):
            ps = acc_pool.tile([128, QT], F32, tag="acc")
            nc.tensor.matmul(
                ps[:, :], ones128[0:1, :], urow[0:1, qt * QT:(qt + 1) * QT],
                start=True, stop=True,
            )
            nc.vector.tensor_scalar(
                out=updated[:, qt * QT:(qt + 1) * QT], in0=ps[:, :],
                scalar1=0.0, scalar2=None, op0=ALU.is_gt,
            )
        nc.vector.memset(updated[:, 0:1], 1.0)  # CLS token always updated

        # ---- PE warm-up: dummy matmuls with no input deps keep the HAM
        # activity monitor busy during the input DMAs so the real matmuls
        # start at 2.4 GHz instead of 1.2 GHz.
        warm = acc_pool.tile([128, QT], F32, tag="acc")
        for i in range(40):
            nc.tensor.matmul(warm[:, 0:128], ones128[:, :], ones128[:, :],
                             start=(i == 0), stop=(i == 39))
        warm_sink = singles.tile([1, 1], F32, tag="warm_sink")
        nc.vector.tensor_copy(warm_sink[0:1, :], warm[0:1, 0:1])
        warm_dram = dram_pool.tile([1], F32, tag="warm_dram")
        nc.gpsimd.dma_start(out=warm_dram[:].rearrange("(a b) -> a b", a=1),
                            in_=warm_sink[0:1, :])

        # ---- mask propagation projection (transposed layout) ----
        # m-chunks 6..8 first (v_m maxes feed a DRAM round-trip whose latency
        # wants to hide under the remaining projections), then 0..5.
        for m in (6, 7, 8, 0, 1, 2, 3, 4, 5):
            strip = strip_pool.tile([128, NQT, 512], F32, tag="strip")
            for qt in range(NQT):
                for kc in range(DC):
                    if m < 6:
                        lhsT = s_wqkTa[:, kc, m * 128:(m + 1) * 128]
                    else:
                        lhsT = s_wvTa[:, kc, (m - 6) * 128:(m - 5) * 128]
                    nc.tensor.matmul(
                        strip[:, qt, 0:QT], lhsT,
                        s_maskT[:, kc, qt * QT:(qt + 1) * QT],
                        start=(kc == 0), stop=(kc == DC - 1),
                    )
            if m < 6:
                # raw copy -> SBUF bf16, then max / reciprocal / in-place norm
                for qt in range(NQT):
                    nc.vector.tensor_copy(
                        qkmT[:, m, qt * QT:(qt + 1) * QT], strip[:, qt, 0:QT])
                rmax = small_pool.tile([128, 1], F32, tag="rmax")
                nc.vector.reduce_max(rmax[:, :], qkmT[:, m, :], axis=AX.X)
                nc.vector.tensor_scalar_add(rmax[:, :], rmax[:, :], 1e-6)
                rrec = small_pool.tile([128, 1], F32, tag="rrec")
                nc.vector.reciprocal(rrec[:, :], rmax[:, :])
                nc.vector.tensor_scalar_mul(qkmT[:, m, :], qkmT[:, m, :], rrec[:, :])
                nc.vector.memset(qkmT[:, m, 0:1], 1.0)  # CLS -> 1
            else:
                rmax = small_pool.tile([128, 1], F32, tag="rmax")
                nc.vector.reduce_max(rmax[:, :], strip[:, :, 0:QT], axis=AX.XY)
                nc.vector.tensor_scalar_add(rmax[:, :], rmax[:, :], 1e-6)
                rrec = small_pool.tile([128, 1], F32, tag="rrec")
                nc.vector.reciprocal(rrec[:, :], rmax[:, :])
                nc.vector.tensor_copy(vrec_cols[:, m - 6:m - 5], rrec[:, :])

        # vrec columns [128, 3] -> row [1, 384] via DRAM round-trip.
        # gpsimd queue: a long semaphore wait here must not block the sync
        # engine's DMA stream.
        nc.gpsimd.dma_start(
            out=vrec_dram[:].rearrange("(m p) -> p m", p=128), in_=vrec_cols[:, :])
        nc.gpsimd.dma_start(
            out=vrec_row[0:1, :], in_=vrec_dram[:].rearrange("(a n) -> a n", a=1))
        psb = acc_pool.tile([128, INNER], F32, tag="acc")
        nc.tensor.matmul(psb[:, :], ones128[0:1, :], vrec_row[0:1, :],
                         start=True, stop=True)
        nc.vector.tensor_copy(bcast_vrec[:, :], psb[:, :])

        # ---- x projection for q, k (transposed layout), fused with q_m/k_m mult ----
        for m in range(6):
            strip = strip_pool.tile([128, NQT, 512], F32, tag="strip")
            for qt in range(NQT):
                for kc in range(DC):
                    nc.tensor.matmul(
                        strip[:, qt, 0:QT], s_wqkT[:, kc, m * 128:(m + 1) * 128],
                        s_xT[:, kc, qt * QT:(qt + 1) * QT],
                        start=(kc == 0), stop=(kc == DC - 1),
                    )
            for qt in range(NQT):
                nc.vector.tensor_mul(
                    qqkkT[:, m, qt * QT:(qt + 1) * QT],
                    strip[:, qt, 0:QT],
                    qkmT[:, m, qt * QT:(qt + 1) * QT],
                )

        # ---- v / v_m in token layout, fused products into vvm ----
        for tg in range(3):
            psV = strip_pool.tile([128, NQT, 512], F32, tag="strip")
            psVm = strip_pool.tile([128, NQT, 512], F32, tag="strip")
            for t3 in range(3):
                tok = tg * 3 + t3
                for kc in range(DC):
                    nc.tensor.matmul(
                        psV[:, t3, 0:INNER], s_xT[:, kc, tok * 128:(tok + 1) * 128],
                        s_wvT[:, kc, :],
                        start=(kc == 0), stop=(kc == DC - 1),
                    )
                for kc in range(DC):
                    nc.tensor.matmul(
                        psVm[:, t3, 0:INNER], s_maskT[:, kc, tok * 128:(tok + 1) * 128],
                        s_wvTa[:, kc, :],
                        start=(kc == 0), stop=(kc == DC - 1),
                    )
            for t3 in range(3):
                tok = tg * 3 + t3
                # vm (normalized) -> odd 64-blocks of vvm
                vm_dst = vvm[:, tok, :].rearrange("p (h two hd) -> p h two hd", two=2, hd=HD)[:, :, 1, :]
                vm_src = psVm[:, t3, 0:INNER].rearrange("p (h hd) -> p h hd", hd=HD)
                rec_v = bcast_vrec[:, :].rearrange("p (h hd) -> p h hd", hd=HD)
                nc.vector.tensor_mul(vm_dst, vm_src, rec_v)
                if tok == 0:
                    # CLS row of v_m is exactly 1
                    nc.vector.memset(vvm[0:1, 0, :].rearrange(
                        "p (h two hd) -> p h two hd", two=2, hd=HD)[:, :, 1, :], 1.0)
                # vv = vm * v -> even 64-blocks of vvm
                vv_dst = vvm[:, tok, :].rearrange("p (h two hd) -> p h two hd", two=2, hd=HD)[:, :, 0, :]
                v_src = psV[:, t3, 0:INNER].rearrange("p (h hd) -> p h hd", hd=HD)
                nc.vector.tensor_mul(vv_dst, vm_dst, v_src)

        # ---- attention per head ----
        for h in range(HEADS):
            hb = (h % 2) * 64          # partition base within chunk
            hc = h // 2                # chunk index
            expT = exp_pool.tile([128, TC, NPAD], BF, tag="expT")
            # last key chunk holds only one real key (token 1024): zero the
            # whole chunk first, then let exp write just row 0.
            nc.vector.memset(expT[:, TC - 1, :], 0.0)
            for kc in range(TC):
                strip = strip_pool.tile([128, NQT, 512], F32, tag="strip")
                for qt in range(NQT):
                    nc.tensor.matmul(
                        strip[:, qt, 0:QT],
                        qqkkT[hb:hb + HD, 3 + hc, kc * 128:(kc + 1) * 128],
                        qqkkT[hb:hb + HD, hc, qt * QT:(qt + 1) * QT],
                        start=True, stop=True,
                    )
                if kc == TC - 1:
                    nc.scalar.activation(
                        out=expT[0:1, kc, :], in_=strip[0:1, :, 0:QT],
                        func=ACTF.Exp, scale=SCALE,
                    )
                else:
                    nc.scalar.activation(
                        out=expT[:, kc, :], in_=strip[:, :, 0:QT],
                        func=ACTF.Exp, scale=SCALE,
                    )

            multb = mult_pool.tile([128, NPAD], BF, tag="multb")
            for qt in range(NQT):
                dacc = acc_pool.tile([128, QT], F32, tag="acc")
                for kc in range(TC):
                    nc.tensor.matmul(
                        dacc[:, :], ones128[:, :],
                        expT[:, kc, qt * QT:(qt + 1) * QT],
                        start=(kc == 0), stop=(kc == TC - 1),
                    )
                # 1/denom = exp(-ln(denom)) on the scalar engine; DVE's
                # iterative reciprocal is ~6.7 ns/elem and stalled the AV
                # accumulation by holding the PSUM slot.
                rf32 = small_pool.tile([128, QT], F32, tag="rf32")
                nc.scalar.activation(out=rf32[:, :], in_=dacc[:, :], func=ACTF.Ln)
                nc.scalar.activation(out=multb[:, qt * QT:(qt + 1) * QT],
                                     in_=rf32[:, :], func=ACTF.Exp, scale=-1.0)
                nc.vector.tensor_mul(
                    multb[:, qt * QT:(qt + 1) * QT],
                    multb[:, qt * QT:(qt + 1) * QT],
                    updated[:, qt * QT:(qt + 1) * QT])
            for qt in range(NQT):
                aacc = acc_pool.tile([128, QT], F32, tag="acc")
                for kc in range(TC):
                    nc.tensor.matmul(
                        aacc[:, :], vvm[:, kc, h * 128:(h + 1) * 128],
                        expT[:, kc, qt * QT:(qt + 1) * QT],
                        start=(kc == 0), stop=(kc == TC - 1),
                    )
                nc.vector.tensor_mul(
                    ohT[hb:hb + HD, hc, qt * QT:(qt + 1) * QT],
                    aacc[0:HD, :], multb[0:HD, qt * QT:(qt + 1) * QT])
                nc.vector.tensor_mul(
                    mhT[hb:hb + HD, hc, qt * QT:(qt + 1) * QT],
                    aacc[HD:128, :], multb[HD:128, qt * QT:(qt + 1) * QT])

        # ---- final projections (partial over this core's 384 inner dims) ----
        outd = d["out"]
        for t, (wmat, src) in enumerate(((s_woT, ohT), (s_woTa, mhT))):
            for m in range(6):
                strip = strip_pool.tile([128, NQT, 512], F32, tag="strip")
                for qt in range(NQT):
                    for ic in range(IC):
                        nc.tensor.matmul(
                            strip[:, qt, 0:QT], wmat[:, ic, m * 128:(m + 1) * 128],
                            src[:, ic, qt * QT:(qt + 1) * QT],
                            start=(ic == 0), stop=(ic == IC - 1),
                        )
                stage = mult_pool.tile([128, NPAD], F32, tag="ostage")
                nc.vector.tensor_copy(stage[:, :].rearrange("p (q n) -> p q n", n=QT),
                                      strip[:, :, 0:QT])
                nc.sync.dma_start(
                    out=outd.ap()[t, m * 128:(m + 1) * 128, :],
                    in_=stage[:, :],
                )


_CACHED = None


def _build():
    global _CACHED
    if _CACHED is not None:
        return _CACHED
    nc = bacc.Bacc("TRN2", target_bir_lowering=False, debug=False, num_devices=8)
    d = {}
    d["xT"] = nc.dram_tensor("xT", [DIM, NPAD], BF, kind="ExternalInput")
    d["maskT"] = nc.dram_tensor("maskT", [DIM, NPAD], BF, kind="ExternalInput")
    d["wqkT"] = nc.dram_tensor("wqkT", [DIM, 768], BF, kind="ExternalInput")
    d["wqkTa"] = nc.dram_tensor("wqkTa", [DIM, 768], BF, kind="ExternalInput")
    d["wvT"] = nc.dram_tensor("wvT", [DIM, INNER], BF, kind="ExternalInput")
    d["wvTa"] = nc.dram_tensor("wvTa", [DIM, INNER], BF, kind="ExternalInput")
    d["woT"] = nc.dram_tensor("woT", [INNER, 768], BF, kind="ExternalInput")
    d["woTa"] = nc.dram_tensor("woTa", [INNER, 768], BF, kind="ExternalInput")
    d["out"] = nc.dram_tensor("out", [2, DIM, NPAD], F32, kind="ExternalOutput")

    with tile.TileContext(nc) as tcx:
        _emit(nc, tcx, d)
    nc.compile()
    _CACHED = nc
    return nc


def _host_prep(x, mask, w_qkv, w_out):
    """Build the 8 per-core input maps (bf16, transposed + padded)."""
    B = x.shape[0]
    in_maps = []
    for core in range(8):
        b, g = core // 2, core % 2
        sl = slice(g * INNER, (g + 1) * INNER)

        xT = np.zeros((DIM, NPAD), dtype=BF16)
        xT[:, :NTOK] = x[b].T.astype(BF16)
        maskT = np.zeros((DIM, NPAD), dtype=BF16)
        maskT[:, 1:NTOK] = mask[b].T.astype(BF16)

        w_q = w_qkv[sl]                      # [384, 768]
        w_k = w_qkv[768 + g * INNER: 768 + (g + 1) * INNER]
        w_v = w_qkv[1536 + g * INNER: 1536 + (g + 1) * INNER]
        wqk = np.concatenate([w_q, w_k], axis=0)          # [768, 768]
        wqkT = np.ascontiguousarray(wqk.T).astype(BF16)
        wqkTa = np.abs(wqkT).astype(BF16)
        wvT = np.ascontiguousarray(w_v.T).astype(BF16)     # [768, 384]
        wvTa = np.abs(wvT).astype(BF16)
        woT = np.ascontiguousarray(w_out[:, sl].T).astype(BF16)   # [384, 768]
        woTa = np.abs(woT).astype(BF16)

        in_maps.append({
            "xT": xT, "maskT": maskT,
            "wqkT": wqkT, "wqkTa": wqkTa,
            "wvT": wvT, "wvTa": wvTa,
            "woT": woT, "woTa": woTa,
        })
    return in_maps


def run(inputs, trace=False):
    """Run on hardware; returns ((out, m), exec_time_ns)."""
    x = np.asarray(inputs["x"], dtype=np.float32)
    mask = np.asarray(inputs["mask"], dtype=np.float32)
    w_qkv = np.asarray(inputs["w_qkv"], dtype=np.float32)
    w_out = np.asarray(inputs["w_out"], dtype=np.float32)
    b_out = np.asarray(inputs["b_out"], dtype=np.float32)

    nc = _build()
    in_maps = _host_prep(x, mask, w_qkv, w_out)
    res = run_bass_kernel_spmd(nc, in_maps, list(range(8)), trace=trace)

    B = x.shape[0]
    outs, ms = [], []
    for b in range(B):
        p0 = np.asarray(res.results[2 * b]["out"], dtype=np.float32)
        p1 = np.asarray(res.results[2 * b + 1]["out"], dtype=np.float32)
        s = p0 + p1                                  # [2, 768, 1152]
        outs.append(s[0].T[:NTOK] + b_out)           # [1025, 768]
        ms.append(s[1].T[1:NTOK])                    # [1024, 768]
    out = np.stack(outs).astype(np.float32)
    m = np.stack(ms).astype(np.float32)
    return (out, m), res.exec_time_ns


def kernel(**inputs):
    (out, m), _ = run(inputs, trace=False)
    return (out, m)
